# revision 14
# baseline (speedup 1.0000x reference)
"""Trainium2 Bass kernel for nn_Attention_7584912245222.

Math (reference):
    hidden = tanh(memory @ Wh + (query @ Wq)[:, None, :])   # [B, T, D]
    s      = softmax(hidden @ v, axis=T)                    # [B, T]
    out    = einsum('btd,bt->bd', memory, s)                # [B, D]

Strategy: pure data-parallel over batch B=64 across 8 NeuronCores
(8 batches per core). Weights replicated. No collectives.

Device pipeline (per core, per batch b):
  - main GEMM computes hidden.T in [e(partitions), t(free)] orientation:
      lhsT = Wh[d,e] tiles (natural layout, stationary)
      rhs  = mem[b].T tiles [d, t] (host pre-transposed, streamed)
    so the per-batch bias qvec[b][e] is a per-partition scalar, fused into
    the PSUM->SBUF tanh activation on ScalarE.
  - v-weighting runs on the (otherwise idle) VectorE:
      acc_hv[p, t] += h_m[p, t] * v[m*128+p]   (scalar_tensor_tensor)
    and the partition reduction s[t] = sum_p acc_hv[p, t] is done with 16
    tiny PE matmuls  lhsT=acc_hv[:, j*128:(j+1)*128], rhs=ones [128,1]
    which produce s directly TRANSPOSED as [t(partitions), 1] columns.
  - softmax without max-subtraction (logits are bounded, |s| < ~4): one
    Exp activation on the [128, KT] transposed tile, accum_out + one
    [128,1] @ ones matmul give the normalizer.
  - final weighted sum also on VectorE:
      acc_d[p, d] += memN_j[p, d] * s_exp[j*128+p]          (fp32 acc)
    followed by a 2-matmul partition reduction, scaled by 1/Z via the
    activation `scale` operand on the way out. (Last batch uses PE
    matmuls instead, to shorten the kernel tail.)

All matmuls bf16 (host pre-cast), fp32 accumulation in PSUM; the output
weighted-sum accumulator is fp32.
"""

import sys

if "/opt/trn_rl_repo" not in sys.path:
    sys.path.insert(0, "/opt/trn_rl_repo")

import numpy as np
import ml_dtypes

import concourse.bass as bass
import concourse.tile as tile
from concourse import bacc, mybir
from concourse.bass_utils import run_bass_kernel_spmd

BF16 = ml_dtypes.bfloat16

# Problem shapes (hardcoded per spec)
B, T, D, Q = 64, 2048, 1024, 1024
N_CORES = 8
BL = B // N_CORES  # batches per core


def build(nc, BL=BL, T=T, D=D, Q=Q):
    """Emit the per-core kernel into `nc`. Returns nc."""
    f32 = mybir.dt.float32
    bf16 = mybir.dt.bfloat16
    AF = mybir.ActivationFunctionType
    ALU = mybir.AluOpType

    P = 128
    TC = min(512, T)          # t-chunk size for the main GEMM
    DC = min(512, D)          # d-chunk size for the final output
    KD = D // P               # d contraction tiles
    ME = D // P               # e output tiles
    KQ = Q // P               # q contraction tiles
    NT = T // TC              # t chunks
    KT = T // P               # t contraction tiles (final sum)
    ND = D // DC              # output d chunks

    memT = nc.declare_dram_parameter("memT", [BL, D, T], bf16, isOutput=False)
    memN = nc.declare_dram_parameter("memN", [BL, T, D], bf16, isOutput=False)
    wh = nc.declare_dram_parameter("Wh", [D, D], bf16, isOutput=False)
    wq = nc.declare_dram_parameter("Wq", [Q, D], bf16, isOutput=False)
    qryT = nc.declare_dram_parameter("qryT", [P, KQ * BL], bf16, isOutput=False)
    vT = nc.declare_dram_parameter("vT", [P, KD], f32, isOutput=False)
    out_ext = nc.declare_dram_parameter("out", [BL, D], f32, isOutput=True)

    with tile.TileContext(nc) as tc:
        from contextlib import ExitStack

        with ExitStack() as ctx:
            const_pool = ctx.enter_context(tc.tile_pool(name="const", bufs=1))

            wh_sb = const_pool.tile([P, KD * D], bf16, tag="wh")
            v_sb = const_pool.tile([P, KD], f32, tag="v")
            ones_sb = const_pool.tile([P, 1], bf16, tag="ones")
            nc.gpsimd.memset(ones_sb[:], 1.0)
            ones32_sb = const_pool.tile([P, 1], f32, tag="ones32")
            nc.gpsimd.memset(ones32_sb[:], 1.0)
            qT_sb = const_pool.tile([P, ME * BL], f32, tag="qT")  # col = m*BL+b

            mT_pool = ctx.enter_context(tc.tile_pool(name="mT", bufs=2))

            def emit_mT_load(b, tiles={}):
                mT_sb = mT_pool.tile([P, KD * T], bf16, tag="mT", name=f"mT{b}")
                nc.scalar.dma_start(
                    mT_sb[:].rearrange("p (k t) -> p k t", k=KD),
                    memT[b].rearrange("(k p) t -> p k t", p=P),
                )
                tiles[b] = mT_sb
                return tiles

            # prefetch the first two batches' memT now: the ScalarE ring's
            # doorbell for these must not queue behind the qT-phase copies
            mT_tiles = emit_mT_load(0)
            if BL > 1:
                emit_mT_load(1, mT_tiles)

            # ---- qT[e, b] = (query @ Wq).T, computed once ----
            # DMA order matters (HWDGE rings are FIFO per issuing engine):
            # qry+wq first so the qT matmuls start early, then wh which
            # gates the main GEMM. memT loads go on the ScalarE ring.
            with (
                tc.tile_pool(name="wqp", bufs=1) as wq_pool,
                tc.tile_pool(name="pqp", bufs=1, space="PSUM") as pq_pool,
            ):
                qry_sb = wq_pool.tile([P, KQ * BL], bf16, tag="qry")
                nc.sync.dma_start(qry_sb[:], qryT[:])
                wq_sb = wq_pool.tile([P, KQ * D], bf16, tag="wq")
                nc.sync.dma_start(
                    wq_sb[:].rearrange("p (k e) -> p k e", k=KQ),
                    wq.rearrange("(k p) e -> p k e", p=P),
                )
                nc.sync.dma_start(
                    wh_sb[:].rearrange("p (k e) -> p k e", k=KD),
                    wh.rearrange("(k p) e -> p k e", p=P),
                )
                nc.sync.dma_start(v_sb[:], vT[:])
                for m in range(ME):
                    pq = pq_pool.tile([P, BL], f32, tag="pq")
                    for k in range(KQ):
                        nc.tensor.matmul(
                            pq[:],
                            lhsT=wq_sb[:, k * D + m * P : k * D + (m + 1) * P],
                            rhs=qry_sb[:, k * BL : (k + 1) * BL],
                            start=(k == 0),
                            stop=(k == KQ - 1),
                        )
                    nc.scalar.copy(qT_sb[:, m * BL : (m + 1) * BL], pq[:])

            ph_pool = ctx.enter_context(tc.tile_pool(name="ph", bufs=4, space="PSUM"))
            pm_pool = ctx.enter_context(tc.tile_pool(name="pm", bufs=1, space="PSUM"))

            mN_pool = ctx.enter_context(tc.tile_pool(name="mN", bufs=2))
            h_pool = ctx.enter_context(tc.tile_pool(name="h", bufs=5))
            acc_pool = ctx.enter_context(tc.tile_pool(name="acc", bufs=2))
            accd_pool = ctx.enter_context(tc.tile_pool(name="accd", bufs=2))
            s_pool = ctx.enter_context(tc.tile_pool(name="s", bufs=2))

            def make_phase3(b, acc_hv, mN_sb, use_pe_finals):
                st = {}

                def emit_reduce_exp():
                    # s (transposed): sT_raw[:, j] = acc_hv[:, j*128:(j+1)*128].T @ ones
                    sT_ps = pm_pool.tile([P, KT], f32, tag="sT", name=f"sT{b}")
                    for j in range(KT):
                        nc.tensor.matmul(
                            sT_ps[:, j : j + 1],
                            lhsT=acc_hv[:, j * P : (j + 1) * P],
                            rhs=ones_sb[:, 0:1],
                            start=True,
                            stop=True,
                        )
                    sT_exp = s_pool.tile([P, KT], f32, tag="sTe", name=f"sTe{b}")
                    partials = s_pool.tile([P, 1], f32, tag="par", name=f"par{b}")
                    nc.scalar.activation(
                        sT_exp[:], sT_ps[:], AF.Exp, accum_out=partials[:]
                    )
                    st["sT_exp"] = sT_exp
                    st["partials"] = partials
                    if use_pe_finals:
                        sT16 = s_pool.tile([P, KT], bf16, tag="sTe16", name=f"sTe16_{b}")
                        nc.vector.tensor_copy(sT16[:], sT_exp[:])
                        st["sT16"] = sT16

                def emit_z_and_acc():
                    zps = pm_pool.tile([1, 1], f32, tag="zps", name=f"zps{b}")
                    nc.tensor.matmul(
                        zps[0:1, 0:1],
                        lhsT=ones32_sb[:, 0:1],
                        rhs=st["partials"][:],
                        start=True,
                        stop=True,
                    )
                    rec = s_pool.tile([1, 1], f32, tag="rec", name=f"rec{b}")
                    nc.vector.reciprocal(rec[0:1, 0:1], zps[0:1, 0:1])
                    st["rec"] = rec
                    if not use_pe_finals:
                        # acc_d[p, d] = sum_j memN_j[p, d] * s_exp[j*128+p]
                        acc_d = accd_pool.tile(
                            [P, D], f32, tag="accd", name=f"accd{b}"
                        )
                        for j in range(KT):
                            if j == 0:
                                nc.vector.tensor_scalar_mul(
                                    acc_d[:],
                                    mN_sb[:, 0:D],
                                    st["sT_exp"][:, 0:1],
                                )
                            else:
                                nc.vector.scalar_tensor_tensor(
                                    acc_d[:],
                                    mN_sb[:, j * D : (j + 1) * D],
                                    st["sT_exp"][:, j : j + 1],
                                    acc_d[:],
                                    op0=ALU.mult,
                                    op1=ALU.add,
                                )
                        st["acc_d"] = acc_d

                def emit_out():
                    ops = pm_pool.tile([1, D], f32, tag="ops", name=f"ops{b}")
                    if use_pe_finals:
                        for n in range(ND):
                            for j in range(KT):
                                nc.tensor.matmul(
                                    ops[0:1, n * DC : (n + 1) * DC],
                                    lhsT=st["sT16"][:, j : j + 1],
                                    rhs=mN_sb[
                                        :, j * D + n * DC : j * D + (n + 1) * DC
                                    ],
                                    start=(j == 0),
                                    stop=(j == KT - 1),
                                    skip_group_check=True,
                                )
                    else:
                        for n in range(ND):
                            nc.tensor.matmul(
                                ops[0:1, n * DC : (n + 1) * DC],
                                lhsT=ones32_sb[:, 0:1],
                                rhs=st["acc_d"][:, n * DC : (n + 1) * DC],
                                start=True,
                                stop=True,
                                skip_group_check=True,
                            )
                    out_row = s_pool.tile([1, D], f32, tag="orow", name=f"orow{b}")
                    for n in range(ND):
                        nc.scalar.activation(
                            out_row[0:1, n * DC : (n + 1) * DC],
                            ops[0:1, n * DC : (n + 1) * DC],
                            AF.Copy,
                            scale=st["rec"][0:1, 0:1],
                        )
                    nc.sync.dma_start(out_ext[b : b + 1, :], out_row[:])

                return [emit_reduce_exp, emit_z_and_acc, emit_out]

            # phase3 pieces of batch b run at checkpoints inside batch b+1
            CHECKPOINTS = {1: 0, 2: 1, 5: 2}  # m -> pending index semantics
            pending = []
            for b in range(BL):
                if b not in mT_tiles:
                    emit_mT_load(b, mT_tiles)
                mT_sb = mT_tiles.pop(b)
                if b + 2 < BL:
                    emit_mT_load(b + 2, mT_tiles)
                mN_sb = mN_pool.tile([P, KT * D], bf16, tag="mN", name=f"mN{b}")

                acc_hv = acc_pool.tile([P, T], bf16, tag="acc", name=f"acc{b}")

                for m in range(ME):
                    h_sb = h_pool.tile([P, T], bf16, tag="h", name=f"h{b}_{m}")
                    for n in range(NT):
                        ph = ph_pool.tile([P, TC], f32, tag="ph", name=f"ph{b}_{m}_{n}")
                        for k in range(KD):
                            nc.tensor.matmul(
                                ph[:],
                                lhsT=wh_sb[:, k * D + m * P : k * D + (m + 1) * P],
                                rhs=mT_sb[:, k * T + n * TC : k * T + (n + 1) * TC],
                                start=(k == 0),
                                stop=(k == KD - 1),
                            )
                        nc.scalar.activation(
                            h_sb[:, n * TC : (n + 1) * TC],
                            ph[:],
                            AF.Tanh,
                            bias=qT_sb[:, m * BL + b : m * BL + b + 1],
                        )
                    # v-weighted accumulation on VectorE
                    if m == 0:
                        nc.vector.tensor_scalar_mul(
                            acc_hv[:], h_sb[:], v_sb[:, 0:1]
                        )
                    else:
                        nc.vector.scalar_tensor_tensor(
                            acc_hv[:],
                            h_sb[:],
                            v_sb[:, m : m + 1],
                            acc_hv[:],
                            op0=ALU.mult,
                            op1=ALU.add,
                        )
                    # memN is first needed by phase-3 (during batch b+1);
                    # defer its DMA so it doesn't compete with the critical
                    # memT/weight loads at startup
                    if m == min(3, ME - 1):
                        nc.sync.dma_start(
                            mN_sb[:].rearrange("p (k d) -> p k d", k=KT),
                            memN[b].rearrange("(k p) d -> p k d", p=P),
                        )
                    # interleave previous batch's phase-3 between dense
                    # main-GEMM blocks
                    if m in CHECKPOINTS and pending:
                        pending.pop(0)()
                # flush any leftover phase-3 pieces (small-ME debug configs)
                for fn in pending:
                    fn()

                pending = make_phase3(b, acc_hv, mN_sb, use_pe_finals=(b == BL - 1))

            for fn in pending:
                fn()

    nc.compile()
    return nc


# ---------------------------------------------------------------------------
# Host side
# ---------------------------------------------------------------------------

_CACHED_NC = None


def _get_nc():
    global _CACHED_NC
    if _CACHED_NC is None:
        nc = bacc.Bacc("TRN2", target_bir_lowering=False, debug=False,
                       num_devices=N_CORES)
        _CACHED_NC = build(nc)
    return _CACHED_NC


def prep_in_maps(memory, query, Wh, Wq, v):
    """Shard + lay out inputs for the 8 cores (host-side transforms only)."""
    P = 128
    KQ = Q // P
    KD = D // P
    Wh_b = np.ascontiguousarray(Wh.astype(BF16))
    Wq_b = np.ascontiguousarray(Wq.astype(BF16))
    vT = np.ascontiguousarray(v[:, 0].reshape(KD, P).T.astype(np.float32))  # [128, KD]
    in_maps = []
    for c in range(N_CORES):
        sl = slice(c * BL, (c + 1) * BL)
        mem_c = memory[sl]
        memT_c = np.ascontiguousarray(
            mem_c.transpose(0, 2, 1).astype(BF16)
        )  # [BL, D, T]
        memN_c = np.ascontiguousarray(mem_c.astype(BF16))  # [BL, T, D]
        # qryT[p, k*BL+b] = query[b, k*128+p]  (exact SBUF layout)
        qryT_c = np.ascontiguousarray(
            query[sl].T.reshape(KQ, P, BL).transpose(1, 0, 2).reshape(P, KQ * BL)
            .astype(BF16)
        )
        in_maps.append(
            {
                "memT": memT_c,
                "memN": memN_c,
                "Wh": Wh_b,
                "Wq": Wq_b,
                "qryT": qryT_c,
                "vT": vT,
            }
        )
    return in_maps


def run(in_maps, trace=False, **kwargs):
    nc = _get_nc()
    return run_bass_kernel_spmd(
        nc, in_maps, list(range(N_CORES)), trace=trace, **kwargs
    )


def kernel(memory, query, Wh, Wq, v):
    in_maps = prep_in_maps(memory, query, Wh, Wq, v)
    res = run(in_maps)
    out = np.concatenate([res.results[c]["out"] for c in range(N_CORES)], axis=0)
    return out.astype(np.float32)


# revision 16
# speedup vs baseline: 1.0075x; 1.0075x over previous
"""Trainium2 Bass kernel for nn_Attention_7584912245222.

Math (reference):
    hidden = tanh(memory @ Wh + (query @ Wq)[:, None, :])   # [B, T, D]
    s      = softmax(hidden @ v, axis=T)                    # [B, T]
    out    = einsum('btd,bt->bd', memory, s)                # [B, D]

Strategy: pure data-parallel over batch B=64 across 8 NeuronCores
(8 batches per core). Weights replicated. No collectives.

Device pipeline (per core, per batch b):
  - main GEMM computes hidden.T in [e(partitions), t(free)] orientation:
      lhsT = Wh[d,e] tiles (natural layout, stationary)
      rhs  = mem[b].T tiles [d, t] (host pre-transposed, streamed)
    so the per-batch bias qvec[b][e] is a per-partition scalar, fused into
    the PSUM->SBUF tanh activation on ScalarE.
  - v-weighting runs on the (otherwise idle) VectorE:
      acc_hv[p, t] += h_m[p, t] * v[m*128+p]   (scalar_tensor_tensor)
    and the partition reduction s[t] = sum_p acc_hv[p, t] is done with 16
    tiny PE matmuls  lhsT=acc_hv[:, j*128:(j+1)*128], rhs=ones [128,1]
    which produce s directly TRANSPOSED as [t(partitions), 1] columns.
  - softmax without max-subtraction (logits are bounded, |s| < ~4): one
    Exp activation on the [128, KT] transposed tile, accum_out + one
    [128,1] @ ones matmul give the normalizer.
  - final weighted sum also on VectorE:
      acc_d[p, d] += memN_j[p, d] * s_exp[j*128+p]          (fp32 acc)
    followed by a 2-matmul partition reduction, scaled by 1/Z via the
    activation `scale` operand on the way out. (Last batch uses PE
    matmuls instead, to shorten the kernel tail.)

All matmuls bf16 (host pre-cast), fp32 accumulation in PSUM; the output
weighted-sum accumulator is fp32.
"""

import sys

if "/opt/trn_rl_repo" not in sys.path:
    sys.path.insert(0, "/opt/trn_rl_repo")

import numpy as np
import ml_dtypes

import concourse.bass as bass
import concourse.tile as tile
from concourse import bacc, mybir
from concourse.bass_utils import run_bass_kernel_spmd

BF16 = ml_dtypes.bfloat16

# Problem shapes (hardcoded per spec)
B, T, D, Q = 64, 2048, 1024, 1024
N_CORES = 8
BL = B // N_CORES  # batches per core


def build(nc, BL=BL, T=T, D=D, Q=Q):
    """Emit the per-core kernel into `nc`. Returns nc."""
    f32 = mybir.dt.float32
    bf16 = mybir.dt.bfloat16
    AF = mybir.ActivationFunctionType
    ALU = mybir.AluOpType

    P = 128
    TC = min(512, T)          # t-chunk size for the main GEMM
    DC = min(512, D)          # d-chunk size for the final output
    KD = D // P               # d contraction tiles
    ME = D // P               # e output tiles
    KQ = Q // P               # q contraction tiles
    NT = T // TC              # t chunks
    KT = T // P               # t contraction tiles (final sum)
    ND = D // DC              # output d chunks

    memT = nc.declare_dram_parameter("memT", [BL, D, T], bf16, isOutput=False)
    memN = nc.declare_dram_parameter("memN", [BL, T, D], bf16, isOutput=False)
    wh = nc.declare_dram_parameter("Wh", [D, D], bf16, isOutput=False)
    wq = nc.declare_dram_parameter("Wq", [Q, D], bf16, isOutput=False)
    qryT = nc.declare_dram_parameter("qryT", [P, KQ * BL], bf16, isOutput=False)
    vT = nc.declare_dram_parameter("vT", [P, KD], f32, isOutput=False)
    out_ext = nc.declare_dram_parameter("out", [BL, D], f32, isOutput=True)

    with tile.TileContext(nc) as tc:
        from contextlib import ExitStack

        with ExitStack() as ctx:
            const_pool = ctx.enter_context(tc.tile_pool(name="const", bufs=1))

            wh_sb = const_pool.tile([P, KD * D], bf16, tag="wh")
            v_sb = const_pool.tile([P, KD], f32, tag="v")
            ones_sb = const_pool.tile([P, 1], bf16, tag="ones")
            nc.gpsimd.memset(ones_sb[:], 1.0)
            ones32_sb = const_pool.tile([P, 1], f32, tag="ones32")
            nc.gpsimd.memset(ones32_sb[:], 1.0)
            qT_sb = const_pool.tile([P, ME * BL], f32, tag="qT")  # col = m*BL+b

            mT_pool = ctx.enter_context(tc.tile_pool(name="mT", bufs=2))

            def emit_mT_load(b, tiles={}):
                mT_sb = mT_pool.tile([P, KD * T], bf16, tag="mT", name=f"mT{b}")
                nc.scalar.dma_start(
                    mT_sb[:].rearrange("p (k t) -> p k t", k=KD),
                    memT[b].rearrange("(k p) t -> p k t", p=P),
                )
                tiles[b] = mT_sb
                return tiles

            # prefetch the first two batches' memT now: the ScalarE ring's
            # doorbell for these must not queue behind the qT-phase copies
            mT_tiles = emit_mT_load(0)
            if BL > 1:
                emit_mT_load(1, mT_tiles)

            # ---- qT[e, b] = (query @ Wq).T, computed once ----
            # DMA order matters (HWDGE rings are FIFO per issuing engine):
            # qry+wq first so the qT matmuls start early, then wh which
            # gates the main GEMM. memT loads go on the ScalarE ring.
            with (
                tc.tile_pool(name="wqp", bufs=1) as wq_pool,
                tc.tile_pool(name="pqp", bufs=1, space="PSUM") as pq_pool,
            ):
                qry_sb = wq_pool.tile([P, KQ * BL], bf16, tag="qry")
                nc.sync.dma_start(qry_sb[:], qryT[:])
                wq_sb = wq_pool.tile([P, KQ * D], bf16, tag="wq")
                nc.sync.dma_start(
                    wq_sb[:].rearrange("p (k e) -> p k e", k=KQ),
                    wq.rearrange("(k p) e -> p k e", p=P),
                )
                nc.sync.dma_start(
                    wh_sb[:].rearrange("p (k e) -> p k e", k=KD),
                    wh.rearrange("(k p) e -> p k e", p=P),
                )
                nc.sync.dma_start(v_sb[:], vT[:])
                for m in range(ME):
                    pq = pq_pool.tile([P, BL], f32, tag="pq")
                    for k in range(KQ):
                        nc.tensor.matmul(
                            pq[:],
                            lhsT=wq_sb[:, k * D + m * P : k * D + (m + 1) * P],
                            rhs=qry_sb[:, k * BL : (k + 1) * BL],
                            start=(k == 0),
                            stop=(k == KQ - 1),
                        )
                    nc.scalar.copy(qT_sb[:, m * BL : (m + 1) * BL], pq[:])

            ph_pool = ctx.enter_context(tc.tile_pool(name="ph", bufs=4, space="PSUM"))
            pm_pool = ctx.enter_context(tc.tile_pool(name="pm", bufs=1, space="PSUM"))

            mN_pool = ctx.enter_context(tc.tile_pool(name="mN", bufs=2))
            h_pool = ctx.enter_context(tc.tile_pool(name="h", bufs=5))
            acc_pool = ctx.enter_context(tc.tile_pool(name="acc", bufs=2))
            accd_pool = ctx.enter_context(tc.tile_pool(name="accd", bufs=2))
            s_pool = ctx.enter_context(tc.tile_pool(name="s", bufs=2))

            def make_phase3(b, acc_hv, mN_sb, use_pe_finals):
                st = {}

                def emit_reduce_exp():
                    # s (transposed): sT_raw[:, j] = acc_hv[:, j*128:(j+1)*128].T @ ones
                    sT_ps = pm_pool.tile([P, KT], f32, tag="sT", name=f"sT{b}")
                    for j in range(KT):
                        nc.tensor.matmul(
                            sT_ps[:, j : j + 1],
                            lhsT=acc_hv[:, j * P : (j + 1) * P],
                            rhs=ones_sb[:, 0:1],
                            start=True,
                            stop=True,
                        )
                    sT_exp = s_pool.tile([P, KT], f32, tag="sTe", name=f"sTe{b}")
                    partials = s_pool.tile([P, 1], f32, tag="par", name=f"par{b}")
                    nc.scalar.activation(
                        sT_exp[:], sT_ps[:], AF.Exp, accum_out=partials[:]
                    )
                    st["sT_exp"] = sT_exp
                    st["partials"] = partials
                    if use_pe_finals:
                        sT16 = s_pool.tile([P, KT], bf16, tag="sTe16", name=f"sTe16_{b}")
                        nc.vector.tensor_copy(sT16[:], sT_exp[:])
                        st["sT16"] = sT16

                def emit_z_and_acc():
                    zps = pm_pool.tile([1, 1], f32, tag="zps", name=f"zps{b}")
                    nc.tensor.matmul(
                        zps[0:1, 0:1],
                        lhsT=ones32_sb[:, 0:1],
                        rhs=st["partials"][:],
                        start=True,
                        stop=True,
                    )
                    rec = s_pool.tile([1, 1], f32, tag="rec", name=f"rec{b}")
                    nc.vector.reciprocal(rec[0:1, 0:1], zps[0:1, 0:1])
                    st["rec"] = rec
                    if not use_pe_finals:
                        # acc_d[p, d] = sum_j memN_j[p, d] * s_exp[j*128+p]
                        acc_d = accd_pool.tile(
                            [P, D], f32, tag="accd", name=f"accd{b}"
                        )
                        for j in range(KT):
                            if j == 0:
                                nc.vector.tensor_scalar_mul(
                                    acc_d[:],
                                    mN_sb[:, 0:D],
                                    st["sT_exp"][:, 0:1],
                                )
                            else:
                                nc.vector.scalar_tensor_tensor(
                                    acc_d[:],
                                    mN_sb[:, j * D : (j + 1) * D],
                                    st["sT_exp"][:, j : j + 1],
                                    acc_d[:],
                                    op0=ALU.mult,
                                    op1=ALU.add,
                                )
                        st["acc_d"] = acc_d

                def emit_out():
                    ops = pm_pool.tile([1, D], f32, tag="ops", name=f"ops{b}")
                    if use_pe_finals:
                        for n in range(ND):
                            for j in range(KT):
                                nc.tensor.matmul(
                                    ops[0:1, n * DC : (n + 1) * DC],
                                    lhsT=st["sT16"][:, j : j + 1],
                                    rhs=mN_sb[
                                        :, j * D + n * DC : j * D + (n + 1) * DC
                                    ],
                                    start=(j == 0),
                                    stop=(j == KT - 1),
                                    skip_group_check=True,
                                )
                    else:
                        for n in range(ND):
                            nc.tensor.matmul(
                                ops[0:1, n * DC : (n + 1) * DC],
                                lhsT=ones32_sb[:, 0:1],
                                rhs=st["acc_d"][:, n * DC : (n + 1) * DC],
                                start=True,
                                stop=True,
                                skip_group_check=True,
                            )
                    out_row = s_pool.tile([1, D], f32, tag="orow", name=f"orow{b}")
                    for n in range(ND):
                        nc.scalar.activation(
                            out_row[0:1, n * DC : (n + 1) * DC],
                            ops[0:1, n * DC : (n + 1) * DC],
                            AF.Copy,
                            scale=st["rec"][0:1, 0:1],
                        )
                    nc.sync.dma_start(out_ext[b : b + 1, :], out_row[:])

                return [emit_reduce_exp, emit_z_and_acc, emit_out]

            # phase3 pieces of batch b run at checkpoints inside batch b+1
            CHECKPOINTS = {1: 0, 2: 1, 5: 2}  # m -> pending index semantics
            pending = []
            for b in range(BL):
                if b not in mT_tiles:
                    emit_mT_load(b, mT_tiles)
                mT_sb = mT_tiles.pop(b)
                mN_sb = mN_pool.tile([P, KT * D], bf16, tag="mN", name=f"mN{b}")

                acc_hv = acc_pool.tile([P, T], bf16, tag="acc", name=f"acc{b}")

                for m in range(ME):
                    h_sb = h_pool.tile([P, T], bf16, tag="h", name=f"h{b}_{m}")
                    for n in range(NT):
                        ph = ph_pool.tile([P, TC], f32, tag="ph", name=f"ph{b}_{m}_{n}")
                        for k in range(KD):
                            nc.tensor.matmul(
                                ph[:],
                                lhsT=wh_sb[:, k * D + m * P : k * D + (m + 1) * P],
                                rhs=mT_sb[:, k * T + n * TC : k * T + (n + 1) * TC],
                                start=(k == 0),
                                stop=(k == KD - 1),
                            )
                        nc.scalar.activation(
                            h_sb[:, n * TC : (n + 1) * TC],
                            ph[:],
                            AF.Tanh,
                            bias=qT_sb[:, m * BL + b : m * BL + b + 1],
                        )
                    # v-weighted accumulation on VectorE
                    if m == 0:
                        nc.vector.tensor_scalar_mul(
                            acc_hv[:], h_sb[:], v_sb[:, 0:1]
                        )
                    else:
                        nc.vector.scalar_tensor_tensor(
                            acc_hv[:],
                            h_sb[:],
                            v_sb[:, m : m + 1],
                            acc_hv[:],
                            op0=ALU.mult,
                            op1=ALU.add,
                        )
                    # memN is first needed by phase-3 (during batch b+1);
                    # defer its DMA so it doesn't compete with the critical
                    # memT/weight loads at startup
                    if m == min(3, ME - 1):
                        nc.sync.dma_start(
                            mN_sb[:].rearrange("p (k d) -> p k d", k=KT),
                            memN[b].rearrange("(k p) d -> p k d", p=P),
                        )
                    # interleave previous batch's phase-3 between dense
                    # main-GEMM blocks
                    if m in CHECKPOINTS and pending:
                        pending.pop(0)()
                # prefetch batch b+2's memT now that batch b's slot is
                # almost free (avoids stalling the ScalarE stream early)
                if b + 2 < BL:
                    emit_mT_load(b + 2, mT_tiles)
                # flush any leftover phase-3 pieces (small-ME debug configs)
                for fn in pending:
                    fn()

                pending = make_phase3(b, acc_hv, mN_sb, use_pe_finals=(b == BL - 1))

            for fn in pending:
                fn()

    nc.compile()
    return nc


# ---------------------------------------------------------------------------
# Host side
# ---------------------------------------------------------------------------

_CACHED_NC = None


def _get_nc():
    global _CACHED_NC
    if _CACHED_NC is None:
        nc = bacc.Bacc("TRN2", target_bir_lowering=False, debug=False,
                       num_devices=N_CORES)
        _CACHED_NC = build(nc)
    return _CACHED_NC


def prep_in_maps(memory, query, Wh, Wq, v):
    """Shard + lay out inputs for the 8 cores (host-side transforms only)."""
    P = 128
    KQ = Q // P
    KD = D // P
    Wh_b = np.ascontiguousarray(Wh.astype(BF16))
    Wq_b = np.ascontiguousarray(Wq.astype(BF16))
    vT = np.ascontiguousarray(v[:, 0].reshape(KD, P).T.astype(np.float32))  # [128, KD]
    in_maps = []
    for c in range(N_CORES):
        sl = slice(c * BL, (c + 1) * BL)
        mem_c = memory[sl]
        memT_c = np.ascontiguousarray(
            mem_c.transpose(0, 2, 1).astype(BF16)
        )  # [BL, D, T]
        memN_c = np.ascontiguousarray(mem_c.astype(BF16))  # [BL, T, D]
        # qryT[p, k*BL+b] = query[b, k*128+p]  (exact SBUF layout)
        qryT_c = np.ascontiguousarray(
            query[sl].T.reshape(KQ, P, BL).transpose(1, 0, 2).reshape(P, KQ * BL)
            .astype(BF16)
        )
        in_maps.append(
            {
                "memT": memT_c,
                "memN": memN_c,
                "Wh": Wh_b,
                "Wq": Wq_b,
                "qryT": qryT_c,
                "vT": vT,
            }
        )
    return in_maps


def run(in_maps, trace=False, **kwargs):
    nc = _get_nc()
    return run_bass_kernel_spmd(
        nc, in_maps, list(range(N_CORES)), trace=trace, **kwargs
    )


def kernel(memory, query, Wh, Wq, v):
    in_maps = prep_in_maps(memory, query, Wh, Wq, v)
    res = run(in_maps)
    out = np.concatenate([res.results[c]["out"] for c in range(N_CORES)], axis=0)
    return out.astype(np.float32)
